# revision 38
# baseline (speedup 1.0000x reference)
"""Trainium2 Bass kernel for CodePredictorAttention (B=2, Q=2048, HID=2048,
HQ=16, HKV=4, D=128, causal, qk-rmsnorm + neox rope, GQA).

Sharding (8 cores): data-parallel over batch (2) x tensor-parallel over head
groups (4). Core c handles batch c//4 and q-heads [4g, 4g+4) with kv-head g,
g = c%4. o_proj is row-parallel; the 4 partial outputs per batch are summed
on the host.

v2: all matmul operands in bf16 (FWL halves LDWEIGHTS; 2x DVE elementwise;
half DMA traffic), causal mask via GPSIMD affine_select on the exp'd diag
tiles (no mask matmuls, no identity), q/k transposes via the DMA transpose
XBAR (no PE transposes, no PSUM transpose banks), msq via one DVE
tensor_tensor_reduce per lane, rstd = sqrt(reciprocal_approx_fast(msq+eps)).

Per-core pipeline:
  1. qkv projection  out[tok, feat] = x^T-tiles.T @ w-tiles  (psum fp32)
  2. evict to bf16 (q on DVE, k/v on ACT); msq per lane on DVE TTR;
     rstd chain DVE recip + ACT sqrt; per-lane scale; neox rope on DVE;
     q/k -> [D, tok] via DMA-transpose XBAR.
  3. attention in S^T layout: S^T[k,q] = kT.T @ qT (fp32 psum), E =
     exp(S^T * scale) on ACT (bf16 out), diag tiles zeroed causally via
     GPSIMD affine_select, O^T[D,q] = V.T @ E and colsums = ones.T @ E
     on PE; normalize O^T = O^T * (1/sums) on DVE.
  4. o_proj out[tok, hid] = O^T-tiles.T @ wo^T-tiles, bf16 out to DRAM
     (host upcasts + sums the 4 row-parallel partials).
"""
import os
import numpy as np
from contextlib import ExitStack

import concourse.bass as bass
import concourse.tile as tile
from concourse import bacc, mybir
from concourse.bass_utils import run_bass_kernel_spmd

import ml_dtypes

BF16NP = ml_dtypes.bfloat16

B, Q, HID = 2, 2048, 2048
HQ, HKV, D = 16, 4, 128
NQH = HQ // HKV          # q heads per core = 4
EPS = 1e-6
THETA = 1000000.0
SCALE = float(D) ** -0.5
P = 128
TOK_T = Q // P           # 16 token tiles
KT = HID // P            # 16 hid contraction tiles
QM = 4                   # q-macro tiles of 512
QMW = Q // QM            # 512
F32 = mybir.dt.float32
BF = mybir.dt.bfloat16
I32 = mybir.dt.int32
AF = mybir.ActivationFunctionType
OP = mybir.AluOpType

RSQRT_MAGIC = 0x5F3759DF

NH = NQH + 1
d2 = D // 2

last_exec_time_ns = None   # set when BASS_TRACE=1
last_results = None        # full BassKernelResults for trace analysis


def _emit(ctx, tc, io, apply_qw, apply_kw):
    nc = tc.nc

    const = ctx.enter_context(tc.tile_pool(name="const", bufs=1))
    xpool = ctx.enter_context(tc.tile_pool(name="xp", bufs=4))
    qkp = ctx.enter_context(tc.tile_pool(name="qkp", bufs=2))
    qknp = ctx.enter_context(tc.tile_pool(name="qknp", bufs=3))
    rsq = ctx.enter_context(tc.tile_pool(name="rsq", bufs=3))
    big = ctx.enter_context(tc.tile_pool(name="big", bufs=1))
    blk = ctx.enter_context(tc.tile_pool(name="blk", bufs=2))
    blko = ctx.enter_context(tc.tile_pool(name="blko", bufs=2))
    epool = ctx.enter_context(tc.tile_pool(name="ep", bufs=6))
    opool = ctx.enter_context(tc.tile_pool(name="op", bufs=3))
    recp = ctx.enter_context(tc.tile_pool(name="recp", bufs=2))
    scrp = ctx.enter_context(tc.tile_pool(name="scrp", bufs=2))
    # PSUM bank map (8 banks):
    #   sps x2 | ops_ x1 | sums x1 | qps x1 | kvps x1 | {tp|pps} shared x2
    ps_s = ctx.enter_context(tc.tile_pool(name="psS", bufs=2, space="PSUM"))
    ps_o = ctx.enter_context(tc.tile_pool(name="psO", bufs=1, space="PSUM"))
    ps_m = ctx.enter_context(tc.tile_pool(name="psM", bufs=1, space="PSUM"))
    ps_q = ctx.enter_context(tc.tile_pool(name="psQ", bufs=1, space="PSUM"))
    ps_kv = ctx.enter_context(tc.tile_pool(name="psKV", bufs=1, space="PSUM"))
    ps_p = ctx.enter_context(tc.tile_pool(name="psP", bufs=2, space="PSUM"))

    # ---- earliest x tiles + w k-slices interleaved on the DMA queue so the
    # first qkv matmuls can start ~2us in (w arrives k-slice by k-slice and
    # tile-0's k-loop streams right behind it) ----
    x_tiles = {}

    def prefetch_x(t, split=2):
        x_sb = xpool.tile([P, KT, P], BF, tag="x", name=f"x{t}")
        step = KT // split
        for kc in range(0, KT, step):
            nc.sync.dma_start(x_sb[:, kc:kc + step, :],
                              io["xt"][:, t, kc:kc + step, :])
        x_tiles[t] = x_sb

    w_sb = const.tile([P, KT, 512 + 2 * P], BF, tag="wbig")
    # first x quarter-tile + w k0 land ~1us in so tile-0's k-loop starts
    # immediately and streams behind the w k-slice DMAs
    # queue plan: sync carries x0 + even w k-slices (tile-0's k-loop streams
    # right behind them); the scalar queue carries odd w k-slices + the small
    # constants, doubling startup weight bandwidth.
    x0 = xpool.tile([P, KT, P], BF, tag="x", name="x0")
    nc.sync.dma_start(x0[:, 0:4, :], io["xt"][:, 0, 0:4, :])
    nc.sync.dma_start(x0[:, 4:KT, :], io["xt"][:, 0, 4:KT, :])
    x_tiles[0] = x0
    for k in range(0, KT, 2):
        nc.sync.dma_start(w_sb[:, k, :], io["wt"][k * P:(k + 1) * P, :])
    for k in range(1, KT, 2):
        nc.scalar.dma_start(w_sb[:, k, :], io["wt"][k * P:(k + 1) * P, :])
    prefetch_x(1)

    cos_sb = const.tile([P, TOK_T, d2], BF)
    nc.scalar.dma_start(cos_sb[:], io["cos"][:])
    sin_sb = const.tile([P, TOK_T, d2], BF)
    nc.scalar.dma_start(sin_sb[:], io["sin"][:])
    ones_sb = const.tile([P, P], BF)
    nc.scalar.dma_start(ones_sb[:], io["ones"][:])
    ident_sb = const.tile([P, P], BF)
    nc.scalar.dma_start(ident_sb[:], io["ident"][:])
    magic_sb = const.tile([P, NH], I32)
    nc.vector.memset(magic_sb[:], RSQRT_MAGIC)
    if apply_qw:
        wqrep_sb = const.tile([P, NQH * P], BF)
        nc.scalar.dma_start(wqrep_sb[:], io["wqrep"][:])
    if apply_kw:
        wkrep_sb = const.tile([P, P], BF)
        nc.scalar.dma_start(wkrep_sb[:], io["wkrep"][:])

    # ---- resident activations ----
    kT_sb = big.tile([P, Q], BF)             # [D, tok]
    v_sb = big.tile([P, TOK_T, D], BF)       # [tok%128, t, D]

    # o_proj weights resident; DMA emitted at start of block j=1 so it does
    # not crowd startup bandwidth
    wo_sb = const.tile([P, NQH, HID], BF)

    sq_scale = 1.0 / float(D)
    qkn_tiles = {}

    def qkv_block(t, qk_blk, qTb):
        """qkv matmuls + rmsnorm + rope for token tile t."""
        tb = t % 4
        if t not in x_tiles:
            prefetch_x(t)
        x_sb = x_tiles.pop(t)
        qps = ps_q.tile([P, NQH * P], F32, tag="q", name=f"qps{t}")
        kvps = ps_kv.tile([P, 2 * P], F32, tag="kv", name=f"kvps{t}")
        for k in range(KT):
            nc.tensor.matmul(qps[:], x_sb[:, k, :], w_sb[:, k, 0:NQH * P],
                             start=(k == 0), stop=(k == KT - 1))
        for k in range(KT):
            nc.tensor.matmul(kvps[:], x_sb[:, k, :], w_sb[:, k, NQH * P:],
                             start=(k == 0), stop=(k == KT - 1))
        if t + 4 < TOK_T:
            prefetch_x(t + 4)

        # evict PSUM (fp32->bf16): ACT is faster per-element on PSUM reads
        qk = qk_blk[:, tb]                   # [P, NH, P] bf16
        nc.scalar.copy(qk[:, 0:NQH, :], qps[:])
        nc.scalar.copy(v_sb[:, t, :], kvps[:, P:2 * P])
        nc.scalar.copy(qk[:, NQH, :], kvps[:, 0:P])

        # mean-square per lane on DVE (one fused mul+reduce per lane)
        msq = rsq.tile([P, NH], F32, tag="msq", name=f"msq{t}")
        for h in range(NH):
            scr = scrp.tile([P, P], BF, tag="scr", name=f"scr{t}_{h}")
            nc.vector.scalar_tensor_tensor(
                out=scr[:], in0=qk[:, h, :], scalar=sq_scale, in1=qk[:, h, :],
                op0=OP.mult, op1=OP.mult, accum_out=msq[:, h:h + 1])
        # rstd = rsqrt(msq+eps) on DVE: bit-trick seed + 1 Newton step
        # (seed err ~3.4% -> ~0.2% after one step; bf16 data dominates)
        msqe = rsq.tile([P, NH], F32, tag="msqe", name=f"msqe{t}")
        nc.vector.tensor_scalar(msqe[:], msq[:], EPS, None, op0=OP.add)
        y = rsq.tile([P, NH], F32, tag="rs_y", name=f"y{t}")
        sh = rsq.tile([P, NH], I32, tag="rs_sh", name=f"sh{t}")
        nc.vector.tensor_scalar(sh[:], msqe[:].bitcast(I32), 1, None,
                                op0=OP.arith_shift_right)
        nc.vector.tensor_sub(y[:].bitcast(I32), magic_sb[:], sh[:])
        a = rsq.tile([P, NH], F32, tag="rs_a", name=f"a{t}")
        nc.vector.tensor_mul(a[:], y[:], y[:])
        nc.vector.tensor_mul(a[:], a[:], msqe[:])
        c = rsq.tile([P, NH], F32, tag="rs_c", name=f"c{t}")
        nc.vector.tensor_scalar(c[:], a[:], -0.5, 1.5, op0=OP.mult, op1=OP.add)
        rstd = rsq.tile([P, NH], F32, tag="rstd", name=f"rstd{t}")
        nc.vector.tensor_mul(rstd[:], y[:], c[:])

        # per-lane rstd scale in place on DVE
        for h in range(NH):
            nc.vector.tensor_scalar(qk[:, h, :], qk[:, h, :],
                                    rstd[:, h:h + 1], None, op0=OP.mult)
        if apply_qw:
            qflat = qk_blk[:, tb, 0:NQH, :].rearrange("p h d -> p (h d)")
            nc.vector.tensor_mul(qflat, qflat, wqrep_sb[:])
        if apply_kw:
            nc.vector.tensor_mul(qk[:, NQH, :], qk[:, NQH, :], wkrep_sb[:])

        # neox rope fused across the 5 lanes
        cosb = cos_sb[:, t:t + 1, :].to_broadcast([P, NH, d2])
        sinb = sin_sb[:, t:t + 1, :].to_broadcast([P, NH, d2])
        qk_n = qknp.tile([P, NH, P], BF, tag="qkn", name=f"qkn{t}")
        t1 = qknp.tile([P, NH, d2], BF, tag="t1", name=f"t1_{t}")
        nc.vector.tensor_mul(qk_n[:, :, 0:d2], qk[:, :, d2:D], sinb)
        nc.vector.tensor_mul(t1[:], qk[:, :, 0:d2], cosb)
        nc.vector.tensor_sub(qk_n[:, :, 0:d2], t1[:], qk_n[:, :, 0:d2])
        nc.vector.tensor_mul(qk_n[:, :, d2:D], qk[:, :, 0:d2], sinb)
        nc.vector.tensor_mul(t1[:], qk[:, :, d2:D], cosb)
        nc.vector.tensor_add(qk_n[:, :, d2:D], t1[:], qk_n[:, :, d2:D])

        qkn_tiles[t] = qk_n

    def tp_block(t, qTb):
        """transpose the 5 rope'd lanes into [D, tok] stores (DVE evicts)"""
        tb = (t % 4) * P
        qk_n = qkn_tiles.pop(t)
        for h in range(NH):
            tp = ps_p.tile([P, P], BF, tag="xp", name=f"tp{t}_{h}")
            nc.tensor.transpose(tp[:], qk_n[:, h, :], ident_sb[:])
            if h < NQH:
                nc.vector.tensor_copy(qTb[:, h, tb:tb + P], tp[:])
            else:
                nc.vector.tensor_copy(kT_sb[:, t * P:(t + 1) * P], tp[:])

    def attn_block(h, j, qTb, otb):
        """causal attention for head h, q-macro j (S^T layout). Key tiles are
        processed in pairs (one 2-bank PSUM chunk, one fused exp for off-diag
        pairs); diagonal tiles go first so their gpsimd mask latency hides
        behind the off-diagonal work."""
        nk = 4 * j + 4
        ops_ = ps_o.tile([P, QMW], F32, tag="o", name=f"ops{h}_{j}")
        sums = ps_m.tile([P, QMW], F32, tag="m", name=f"sums{h}_{j}")

        def s_off(i):
            return max(0, (i - 4 * j)) * P

        order = list(range(4 * j, nk)) + list(range(0, 4 * j))

        def s_mm(i):
            off = s_off(i)
            sps = ps_s.tile([P, QMW], F32, tag="s", name=f"sps{h}_{j}_{i}")
            nc.tensor.matmul(sps[:, off:], kT_sb[:, i * P:(i + 1) * P],
                             qTb[:, h, off:], start=True, stop=True)
            return sps

        sps = s_mm(order[0])
        for cnt, i in enumerate(order):
            off = s_off(i)
            e = epool.tile([P, QMW], BF, tag="e", name=f"e{h}_{j}_{i}")
            nc.scalar.activation(e[:, off:], sps[:, off:], AF.Exp, scale=SCALE)
            if i >= 4 * j:
                # zero the q < key part of the diagonal tile
                nc.gpsimd.affine_select(
                    out=e[:, off:off + P], in_=e[:, off:off + P],
                    compare_op=OP.is_ge, fill=0.0,
                    base=0, pattern=[[1, P]], channel_multiplier=-1)
            if cnt + 1 < nk:
                sps = s_mm(order[cnt + 1])
            nc.tensor.matmul(ops_[:, off:], v_sb[:, i, :], e[:, off:],
                             start=(cnt == 0), stop=(cnt == nk - 1))
            nc.tensor.matmul(sums[:, off:], ones_sb[:], e[:, off:],
                             start=(cnt == 0), stop=(cnt == nk - 1))
        # evict O^T to SBUF on ACT first: releases the single ops_ PSUM bank
        # ~0.6us after the last AV instead of after the DVE rec+mul chain,
        # so the next head's first AV/sums matmuls are not blocked on DVE.
        o_sb = recp.tile([P, QMW], BF, tag="osb", name=f"osb{h}_{j}")
        nc.scalar.copy(o_sb[:], ops_[:])
        rec = recp.tile([P, QMW], F32, tag="rec", name=f"rec{h}_{j}")
        nc.vector.reciprocal_approx_fast(out=rec[:], in_=sums[:])
        nc.vector.tensor_mul(otb[:, h, :], o_sb[:], rec[:])

    def oproj_block(t, otb, last=False):
        tb = (t % 4) * P
        for nh in range(NQH):
            pps = ps_p.tile([P, QMW], F32, tag="xp", name=f"pps{t}_{nh}")
            for kf in range(NQH):
                nc.tensor.matmul(pps[:], otb[:, kf, tb:tb + P],
                                 wo_sb[:, kf, nh * QMW:(nh + 1) * QMW],
                                 start=(kf == 0), stop=(kf == NQH - 1))
            o_t = opool.tile([P, QMW], BF, tag="oo", name=f"ot{t}_{nh}")
            if nh % 2 == 0:
                nc.vector.tensor_copy(o_t[:], pps[:])
            else:
                nc.scalar.copy(o_t[:], pps[:])
            # spread the drain of the final block across both DMA queues
            q = nc.scalar if (last and nh % 2 == 1) else nc.sync
            q.dma_start(
                io["out"][t * P:(t + 1) * P, nh * QMW:(nh + 1) * QMW], o_t[:])

    # ======= software-pipelined schedule =======
    # Block j's qkv/norm work is interleaved (in each engine's static order)
    # with block j-1's attention + o_proj so the PE never waits on the serial
    # ACT/DVE norm chain.
    for t0 in range(2, 4):
        prefetch_x(t0)
    qk_blks, qTbs, otbs = {}, {}, {}
    pending_tp = []
    LAG = 1
    for slot in range(QM + LAG):
        j = slot            # qkv block index
        ja = slot - LAG     # attention/oproj block index
        if j == 1:
            for kf in range(NQH):
                nc.sync.dma_start(wo_sb[:, kf, :],
                                  io["wot"][kf * P:(kf + 1) * P, :])
        if j < QM:
            qk_blks[j] = qkp.tile([P, 4, NH, P], BF, tag="qkb",
                                  name=f"qkb{j}")
            qTbs[j] = blk.tile([P, NQH, QMW], BF, tag="qtb", name=f"qTb{j}")
            otbs[j] = blko.tile([P, NQH, QMW], BF, tag="otb", name=f"otb{j}")
        for step in range(4):
            t = 4 * j + step
            if j < QM:
                qkv_block(t, qk_blks[j], qTbs[j])
            # transposes for the previous tile: one-step delay hides the
            # DVE norm/rope chain latency behind the next tile's matmuls.
            if pending_tp and pending_tp[0][0] < t:
                pt, pb = pending_tp.pop(0)
                tp_block(pt, pb)
            if ja >= 0:
                attn_block(step, ja, qTbs[ja], otbs[ja])
            if j < QM:
                pending_tp.append((t, qTbs[j]))
        if j == QM - 1:
            while pending_tp:
                pt, pb = pending_tp.pop(0)
                tp_block(pt, pb)
        if ja >= 0:
            for t2 in range(4 * ja, 4 * ja + 4):
                oproj_block(t2, otbs[ja], last=(ja == QM - 1))
            del qTbs[ja], otbs[ja], qk_blks[ja]


_cache = {}


def _build(apply_qw, apply_kw):
    key = (apply_qw, apply_kw)
    if key in _cache:
        return _cache[key]
    nc = bacc.Bacc("TRN2", target_bir_lowering=False, debug=False)
    io = {
        "xt": nc.dram_tensor("xt", (P, TOK_T, KT, P), BF, kind="ExternalInput")[:],
        "wt": nc.dram_tensor("wt", (HID, 512 + 2 * P), BF, kind="ExternalInput")[:],
        "wot": nc.dram_tensor("wot", (NQH * P, HID), BF, kind="ExternalInput")[:],
        "cos": nc.dram_tensor("cos", (P, TOK_T, d2), BF, kind="ExternalInput")[:],
        "sin": nc.dram_tensor("sin", (P, TOK_T, d2), BF, kind="ExternalInput")[:],
        "ones": nc.dram_tensor("ones", (P, P), BF, kind="ExternalInput")[:],
        "ident": nc.dram_tensor("ident", (P, P), BF, kind="ExternalInput")[:],
        "out": nc.dram_tensor("out", (Q, HID), BF, kind="ExternalOutput")[:],
    }
    if apply_qw:
        io["wqrep"] = nc.dram_tensor("wqrep", (P, NQH * P), BF,
                                     kind="ExternalInput")[:]
    if apply_kw:
        io["wkrep"] = nc.dram_tensor("wkrep", (P, P), BF,
                                     kind="ExternalInput")[:]
    with tile.TileContext(nc) as tc:
        with ExitStack() as ctx:
            _emit(ctx, tc, io, apply_qw, apply_kw)
    nc.compile()
    _cache[key] = nc
    return nc


def kernel(positions, hidden_states, k_cache, v_cache, wqkv, wo, q_norm_w,
           k_norm_w, seq_len):
    global last_exec_time_ns, last_results
    positions = np.asarray(positions)
    hidden_states = np.asarray(hidden_states, dtype=np.float32)
    wqkv = np.asarray(wqkv, dtype=np.float32)
    wo = np.asarray(wo, dtype=np.float32)
    q_norm_w = np.asarray(q_norm_w, dtype=np.float32)
    k_norm_w = np.asarray(k_norm_w, dtype=np.float32)
    if int(np.asarray(seq_len)) != Q:
        raise NotImplementedError("kernel compiled for seq_len == qlen == 2048")

    apply_qw = not np.all(q_norm_w == 1.0)
    apply_kw = not np.all(k_norm_w == 1.0)
    nc = _build(apply_qw, apply_kw)

    # rope tables per batch (fp32 host math, cast bf16)
    inv_freq = 1.0 / (np.float32(THETA) **
                      (np.arange(0, D, 2, dtype=np.float32) / np.float32(D)))
    ones = np.ones((P, P), dtype=BF16NP)
    ident = np.eye(P, dtype=np.float32).astype(BF16NP)

    in_maps = []
    for c in range(8):
        b, g = c // 4, c % 4
        # pre-tiled x^T: xr[p, t, kt, m] = hidden[b][t*128+m, kt*128+p]
        xt = np.ascontiguousarray(
            hidden_states[b].T.reshape(KT, P, TOK_T, P).transpose(1, 2, 0, 3)
        ).astype(BF16NP)
        wq = wqkv[512 * g:512 * (g + 1)]
        wk = wqkv[HQ * D + P * g: HQ * D + P * (g + 1)]
        wv = wqkv[HQ * D + HKV * D + P * g: HQ * D + HKV * D + P * (g + 1)]
        wt = np.ascontiguousarray(
            np.concatenate([wq, wk, wv], axis=0).T).astype(BF16NP)
        wot = np.ascontiguousarray(
            wo[:, 512 * g:512 * (g + 1)].T).astype(BF16NP)
        freqs = positions[b].astype(np.float32)[:, None] * inv_freq[None, :]
        cosf = np.cos(freqs).astype(np.float32)
        sinf = np.sin(freqs).astype(np.float32)
        cosr = np.ascontiguousarray(
            cosf.reshape(TOK_T, P, d2).transpose(1, 0, 2)).astype(BF16NP)
        sinr = np.ascontiguousarray(
            sinf.reshape(TOK_T, P, d2).transpose(1, 0, 2)).astype(BF16NP)
        m = {
            "xt": xt, "wt": wt, "wot": wot,
            "cos": cosr, "sin": sinr, "ones": ones, "ident": ident,
        }
        if apply_qw:
            m["wqrep"] = np.broadcast_to(
                np.tile(q_norm_w, NQH)[None, :],
                (P, NQH * P)).astype(BF16NP).copy()
        if apply_kw:
            m["wkrep"] = np.broadcast_to(
                k_norm_w[None, :], (P, P)).astype(BF16NP).copy()
        in_maps.append(m)

    trace = bool(os.environ.get("BASS_TRACE"))
    res = run_bass_kernel_spmd(nc, in_maps, core_ids=list(range(8)),
                               trace=trace)
    last_exec_time_ns = res.exec_time_ns
    last_results = res

    out = np.empty((B, Q, HID), dtype=np.float32)
    for b in range(B):
        acc = res.results[4 * b]["out"].astype(np.float32)
        for g in range(1, 4):
            acc = acc + res.results[4 * b + g]["out"].astype(np.float32)
        out[b] = acc
    return out
